# revision 3
# baseline (speedup 1.0000x reference)
"""Differential attention kernel for 8 Trainium2 NeuronCores.

Problem: B=2, T=2048, D=2048, H=16 heads of d_head=128 split into two
64-dim sub-heads; dual softmax attention maps combined as A1 - sigmoid(
lambda)*A2, then output projection.

Sharding: core c handles batch b = c//4 and head group hg = c%4 (4 heads).
Each core computes Q/K/V projections for its 4 heads from x[b], runs both
attention maps per head, and produces a partial output projection
out_part = ho @ W_o[:, hg_slice].T.  Host sums the 4 partials per batch.

Device layout choices (everything bf16 except softmax math, fp32 PSUM):
  - All matmul operands transposed on HOST so contraction dims land on
    SBUF partitions: xT=[k,t], wqT/wkT=[k,dq], wvT=[k,dv], woT=[dv,m].
  - Q^T/K^T computed head-major [d_head, T]: sub-head rows 0:64 / 64:128
    feed row-tiled concurrent K=64 score matmuls.
  - Scores computed transposed S^T=[s,t] so softmax'd E^T feeds the
    A@V matmul directly (no transposes anywhere on device).
  - Softmax denominators via ones-vector M=1 matmuls (partition-dim
    reduction on PE); division deferred to after A@V: out^T = P1^T*r1 -
    P2^T*(lam*r2), with per-column r broadcast via K=1 matmuls.
  - Softmax without max-subtraction: scores/8 are within +-6 for this
    distribution; exp stays comfortably in fp32 range.
"""
import sys

sys.path.insert(0, "/opt/trn_rl_repo")

import numpy as np
import ml_dtypes

import concourse.bacc as bacc
import concourse.mybir as mybir
import concourse.tile as tile
from concourse.bass_utils import run_bass_kernel_spmd

F32 = mybir.dt.float32
BF16 = mybir.dt.bfloat16
ALU = mybir.AluOpType
EXP = mybir.ActivationFunctionType.Exp

B, T, D, H = 2, 2048, 2048, 16
DH, DS = 128, 64          # head dim, sub-head dim
NCORES = 8
HPC = 4                   # heads per core
HD = HPC * DH             # 512: head-group width
KC = D // 128             # 16 contraction chunks
TG = 4                    # t-groups of 512
SC = T // 128             # 16 s-chunks
INV_SCALE = 1.0 / 8.0     # 1/sqrt(DS)

_nc_cache = []
last_result = None  # BassKernelResults of the most recent run (for test.py)


def _build():
    nc = bacc.Bacc("TRN2", target_bir_lowering=False, debug=False)
    xT = nc.dram_tensor("xT", [D, T], BF16, kind="ExternalInput")
    wqT = nc.dram_tensor("wqT", [D, HD], BF16, kind="ExternalInput")
    wkT = nc.dram_tensor("wkT", [D, HD], BF16, kind="ExternalInput")
    wvT = nc.dram_tensor("wvT", [D, HD], BF16, kind="ExternalInput")
    woT = nc.dram_tensor("woT", [HD, D], BF16, kind="ExternalInput")
    lamb = nc.dram_tensor("lamb", [1, HD], F32, kind="ExternalInput")
    out = nc.dram_tensor("out", [T, D], F32, kind="ExternalOutput")

    with tile.TileContext(nc) as tc:
        with tc.tile_pool(name="static", bufs=1) as st:
            # persistent operands
            qt = [st.tile([128, T], BF16, name=f"qt{h}") for h in range(HPC)]
            kt = [st.tile([128, T], BF16, name=f"kt{h}") for h in range(HPC)]
            vt = [st.tile([128, HD], BF16, name=f"vt{t}") for t in range(SC)]
            ones_bf = st.tile([128, 1], BF16, name="ones_bf")
            ones_f32 = st.tile([1, 128], F32, name="ones_f32")
            lamb_sb = st.tile([1, HD], F32, name="lamb_sb")
            nc.vector.memset(ones_bf[:], 1.0)
            nc.vector.memset(ones_f32[:], 1.0)
            nc.sync.dma_start(lamb_sb[:], lamb[:])

            # ---------------- projections ----------------
            with tc.tile_pool(name="proj", bufs=1) as pj, \
                 tc.tile_pool(name="psA", bufs=1, space="PSUM") as psA:
                xt = []
                wq = []
                wk = []
                wv = []
                for k in range(KC):
                    xk = pj.tile([128, T], BF16, name=f"xt{k}")
                    nc.sync.dma_start(xk[:], xT[k * 128:(k + 1) * 128, :])
                    xt.append(xk)
                    for nm, lst, dram in (("wq", wq, wqT), ("wk", wk, wkT),
                                          ("wv", wv, wvT)):
                        wt = pj.tile([128, HD], BF16, name=f"{nm}{k}")
                        nc.sync.dma_start(wt[:], dram[k * 128:(k + 1) * 128, :])
                        lst.append(wt)

                # Q^T / K^T per head: [dq=128, T]
                for h in range(HPC):
                    for g in range(TG):
                        tsl = slice(g * 512, (g + 1) * 512)
                        for dst, w in ((qt, wq), (kt, wk)):
                            ps = psA.tile([128, 512], F32, tag="qkps", bufs=4)
                            for k in range(KC):
                                nc.tensor.matmul(
                                    ps[:], w[k][:, h * 128:(h + 1) * 128],
                                    xt[k][:, tsl],
                                    start=(k == 0), stop=(k == KC - 1))
                            nc.vector.tensor_copy(dst[h][:, tsl], ps[:])

                # V s-major: [t=128, dv=512] per t-chunk
                for t in range(SC):
                    ps = psA.tile([128, HD], F32, tag="vps", bufs=4)
                    for k in range(KC):
                        nc.tensor.matmul(ps[:], xt[k][:, t * 128:(t + 1) * 128],
                                         wv[k][:],
                                         start=(k == 0), stop=(k == KC - 1))
                    nc.vector.tensor_copy(vt[t][:], ps[:])

            # ---------------- attention + output projection ----------------
            with tc.tile_pool(name="attn", bufs=1) as at, \
                 tc.tile_pool(name="psB", bufs=1, space="PSUM") as psB:
                wo = []
                for c in range(HPC):
                    woc = at.tile([128, T], BF16, name=f"wo{c}")
                    nc.sync.dma_start(woc[:], woT[c * 128:(c + 1) * 128, :])
                    wo.append(woc)
                ho = [at.tile([128, T], BF16, name=f"ho{h}") for h in range(HPC)]

                for h in range(HPC):
                    hsl = slice(h * 128, (h + 1) * 128)
                    for g in range(TG):
                        tsl = slice(g * 512, (g + 1) * 512)
                        e1l, e2l = [], []
                        for sp in range(SC // 2):  # s-chunk pairs
                            s1 = psB.tile([128, 1024], F32, tag="s1")
                            s2 = psB.tile([128, 1024], F32, tag="s2")
                            for hf in range(2):
                                ssl = slice((2 * sp + hf) * 128,
                                            (2 * sp + hf + 1) * 128)
                                osl = slice(hf * 512, (hf + 1) * 512)
                                nc.tensor.matmul(s1[:, osl], kt[h][0:64, ssl],
                                                 qt[h][0:64, tsl],
                                                 start=True, stop=True)
                                nc.tensor.matmul(s2[:, osl], kt[h][64:128, ssl],
                                                 qt[h][64:128, tsl],
                                                 start=True, stop=True)
                            e1 = at.tile([128, 1024], BF16, tag="e1", bufs=10)
                            e2 = at.tile([128, 1024], BF16, tag="e2", bufs=10)
                            nc.scalar.activation(e1[:], s1[:], EXP,
                                                 scale=INV_SCALE)
                            nc.scalar.activation(e2[:], s2[:], EXP,
                                                 scale=INV_SCALE)
                            e1l.append(e1)
                            e2l.append(e2)

                        p1 = psB.tile([128, 512], F32, tag="p1")
                        p2 = psB.tile([128, 512], F32, tag="p2")
                        sm1 = psB.tile([1, 512], F32, tag="smr", bufs=2)
                        sm2 = psB.tile([1, 512], F32, tag="smr", bufs=2)
                        for sp in range(SC // 2):
                            for hf in range(2):
                                s = 2 * sp + hf
                                osl = slice(hf * 512, (hf + 1) * 512)
                                st_, sp_ = (s == 0), (s == SC - 1)
                                nc.tensor.matmul(p1[:], vt[s][:, hsl],
                                                 e1l[sp][:, osl],
                                                 start=st_, stop=sp_)
                                nc.tensor.matmul(p2[:], vt[s][:, hsl],
                                                 e2l[sp][:, osl],
                                                 start=st_, stop=sp_)
                                nc.tensor.matmul(sm1[:], ones_bf[:],
                                                 e1l[sp][:, osl],
                                                 start=st_, stop=sp_)
                                nc.tensor.matmul(sm2[:], ones_bf[:],
                                                 e2l[sp][:, osl],
                                                 start=st_, stop=sp_)

                        rc1 = at.tile([1, 512], F32, tag="rc1", bufs=2)
                        rc2 = at.tile([1, 512], F32, tag="rc2", bufs=2)
                        nc.vector.reciprocal(rc1[:], sm1[:])
                        nc.vector.reciprocal(rc2[:], sm2[:])
                        r1 = psB.tile([128, 512], F32, tag="smr", bufs=2)
                        r2 = psB.tile([128, 512], F32, tag="smr", bufs=2)
                        nc.tensor.matmul(r1[:], ones_f32[:], rc1[:],
                                         start=True, stop=True)
                        nc.tensor.matmul(r2[:], lamb_sb[0:1, hsl], rc2[:],
                                         start=True, stop=True)
                        r1s = at.tile([128, 512], F32, tag="r1s", bufs=2)
                        r2s = at.tile([128, 512], F32, tag="r2s", bufs=2)
                        nc.vector.tensor_copy(r1s[:], r1[:])
                        nc.vector.tensor_copy(r2s[:], r2[:])
                        tm1 = at.tile([128, 512], F32, tag="tm1", bufs=2)
                        tm2 = at.tile([128, 512], F32, tag="tm2", bufs=2)
                        nc.vector.tensor_mul(tm1[:], p1[:], r1s[:])
                        nc.vector.tensor_mul(tm2[:], p2[:], r2s[:])
                        nc.vector.tensor_sub(ho[h][:, tsl], tm1[:], tm2[:])

                # output projection: out_part[t, m] per [128, 512] tile
                otags = ["s1", "s2", "p1", "p2"]
                for t in range(SC):
                    for mg in range(TG):
                        po = psB.tile([128, 512], F32, tag=otags[mg])
                        for c in range(HPC):
                            nc.tensor.matmul(
                                po[:], ho[c][:, t * 128:(t + 1) * 128],
                                wo[c][:, mg * 512:(mg + 1) * 512],
                                start=(c == 0), stop=(c == HPC - 1))
                        ost = at.tile([128, 512], F32, tag="ost", bufs=4)
                        nc.vector.tensor_copy(ost[:], po[:])
                        nc.sync.dma_start(
                            out[t * 128:(t + 1) * 128,
                                mg * 512:(mg + 1) * 512], ost[:])

    nc.compile()
    return nc


def kernel(x, W_q, W_k, W_v, W_o, lambda_param):
    x = np.asarray(x, dtype=np.float32)
    W_q = np.asarray(W_q, dtype=np.float32)
    W_k = np.asarray(W_k, dtype=np.float32)
    W_v = np.asarray(W_v, dtype=np.float32)
    W_o = np.asarray(W_o, dtype=np.float32)
    lambda_param = np.asarray(lambda_param, dtype=np.float32)

    bf = ml_dtypes.bfloat16
    lam = 1.0 / (1.0 + np.exp(-lambda_param))  # sigmoid, [H]

    in_maps = []
    for c in range(NCORES):
        b, hg = c // HPC, c % HPC
        hs = hg * HD
        in_maps.append({
            "xT": np.ascontiguousarray(x[b].T).astype(bf),
            "wqT": np.ascontiguousarray(W_q[hs:hs + HD, :].T).astype(bf),
            "wkT": np.ascontiguousarray(W_k[hs:hs + HD, :].T).astype(bf),
            "wvT": np.ascontiguousarray(W_v[hs:hs + HD, :].T).astype(bf),
            "woT": np.ascontiguousarray(W_o[:, hs:hs + HD].T).astype(bf),
            "lamb": np.repeat(lam[hs // DH:hs // DH + HPC], DH)
                      .reshape(1, HD).astype(np.float32),
        })

    if not _nc_cache:
        _nc_cache.append(_build())
    nc = _nc_cache[0]

    res = run_bass_kernel_spmd(nc, in_maps, core_ids=list(range(NCORES)))
    global last_result
    last_result = res
    outp = np.zeros((B, T, D), dtype=np.float32)
    for c in range(NCORES):
        outp[c // HPC] += res.results[c]["out"]
    return outp


# revision 7
# speedup vs baseline: 1.3868x; 1.3868x over previous
"""Differential attention kernel for 8 Trainium2 NeuronCores.

Problem: B=2, T=2048, D=2048, H=16 heads of d_head=128 split into two
64-dim sub-heads; dual softmax attention maps combined as A1 - sigmoid(
lambda)*A2, then output projection.

Sharding: core c handles batch b = c//4 and head group hg = c%4 (4 heads).
Each core computes Q/K/V projections for its 4 heads from x[b], runs both
attention maps per head, and produces a partial output projection
out_part = ho @ W_o[:, hg_slice].T.  Host sums the 4 partials per batch.

Device layout choices (everything bf16 except softmax math, fp32 PSUM):
  - All matmul operands transposed on HOST so contraction dims land on
    SBUF partitions: xT=[k,t], wqT/wkT=[k,dq], wvT=[k,dv], woT=[dv,m].
  - Q^T/K^T computed head-major [d_head, T]: sub-head rows 0:64 / 64:128
    feed row-tiled concurrent K=64 score matmuls.
  - Scores computed transposed S^T=[s,t] so softmax'd E^T feeds the
    A@V matmul directly (no transposes anywhere on device).
  - Softmax denominators via ones-vector M=1 matmuls (partition-dim
    reduction on PE); division deferred to after A@V: out^T = P1^T*r1 -
    P2^T*(lam*r2), with per-column r broadcast via K=1 matmuls.
  - Softmax without max-subtraction: scores/8 are within +-6 for this
    distribution; exp stays comfortably in fp32 range.
"""
import sys

sys.path.insert(0, "/opt/trn_rl_repo")

import numpy as np
import ml_dtypes

import concourse.bacc as bacc
import concourse.mybir as mybir
import concourse.tile as tile
from concourse.bass_utils import run_bass_kernel_spmd

# Content-addressed NEFF cache: walrus on this program takes minutes; the
# BIR bytes fully determine the NEFF, so cache across processes.
try:
    import hashlib
    import os as _os
    import pathlib
    import shutil as _sh

    import concourse.bass2jax as _b2j
    import concourse.bass_utils as _bu

    _NEFF_CACHE = pathlib.Path(_os.environ.get("NEFF_CACHE_DIR",
                                               "/tmp/neff_cache"))
    _NEFF_CACHE.mkdir(parents=True, exist_ok=True)
    _orig_cbk = _bu.compile_bir_kernel

    def _cached_cbk(bir_json, tmpdir, neff_name="file.neff"):
        h = hashlib.sha256(bir_json).hexdigest()[:32]
        hit = _NEFF_CACHE / f"{h}_{neff_name}"
        if hit.exists():
            sg = _os.path.join(tmpdir, "sg00")
            _os.makedirs(sg, exist_ok=True)
            dst = _os.path.join(sg, neff_name)
            _sh.copy(hit, dst)
            return dst
        p = _orig_cbk(bir_json, tmpdir, neff_name)
        try:
            _sh.copy(p, hit)
        except OSError:
            pass
        return p

    _bu.compile_bir_kernel = _cached_cbk
    _b2j.compile_bir_kernel = _cached_cbk
except Exception:
    pass

F32 = mybir.dt.float32
BF16 = mybir.dt.bfloat16
ALU = mybir.AluOpType
EXP = mybir.ActivationFunctionType.Exp

B, T, D, H = 2, 2048, 2048, 16
DH, DS = 128, 64          # head dim, sub-head dim
NCORES = 8
HPC = 4                   # heads per core
HD = HPC * DH             # 512: head-group width
KC = D // 128             # 16 contraction chunks
TG = 4                    # t-groups of 512
SC = T // 128             # 16 s-chunks
INV_SCALE = 1.0 / 8.0     # 1/sqrt(DS)

_nc_cache = []
last_result = None  # BassKernelResults of the most recent run (for test.py)


def _build():
    nc = bacc.Bacc("TRN2", target_bir_lowering=False, debug=False)
    xT = nc.dram_tensor("xT", [D, T], BF16, kind="ExternalInput")
    wqT = nc.dram_tensor("wqT", [D, HD], BF16, kind="ExternalInput")
    wkT = nc.dram_tensor("wkT", [D, HD], BF16, kind="ExternalInput")
    wvT = nc.dram_tensor("wvT", [D, HD], BF16, kind="ExternalInput")
    woT = nc.dram_tensor("woT", [HD, D], BF16, kind="ExternalInput")
    lamb = nc.dram_tensor("lamb", [1, HD], F32, kind="ExternalInput")
    out = nc.dram_tensor("out", [T, D], F32, kind="ExternalOutput")

    with tile.TileContext(nc) as tc:
        with tc.tile_pool(name="static", bufs=1) as st:
            # persistent operands
            qt = [st.tile([128, T], BF16, name=f"qt{h}") for h in range(HPC)]
            kt = [st.tile([128, T], BF16, name=f"kt{h}") for h in range(HPC)]
            vt = [st.tile([128, HD], BF16, name=f"vt{t}") for t in range(SC)]
            ones_bf = st.tile([128, 1], BF16, name="ones_bf")
            ones_row = st.tile([1, 128], BF16, name="ones_row")
            lamb_sb = st.tile([1, HD], F32, name="lamb_sb")
            nc.vector.memset(ones_bf[:], 1.0)
            nc.vector.memset(ones_row[:], 1.0)
            nc.sync.dma_start(lamb_sb[:], lamb[:])

            # ---------------- projections ----------------
            with tc.tile_pool(name="proj", bufs=1) as pj, \
                 tc.tile_pool(name="psA", bufs=1, space="PSUM") as psA:
                xt = []
                wq = []
                wk = []
                wv = []
                for k in range(KC):
                    xk = pj.tile([128, T], BF16, name=f"xt{k}")
                    nc.sync.dma_start(xk[:], xT[k * 128:(k + 1) * 128, :])
                    xt.append(xk)
                    for nm, lst, dram in (("wq", wq, wqT), ("wk", wk, wkT),
                                          ("wv", wv, wvT)):
                        wt = pj.tile([128, HD], BF16, name=f"{nm}{k}")
                        nc.sync.dma_start(wt[:], dram[k * 128:(k + 1) * 128, :])
                        lst.append(wt)

                # Q^T / K^T per head: [dq=128, T]
                for h in range(HPC):
                    for g in range(TG):
                        tsl = slice(g * 512, (g + 1) * 512)
                        for dst, w in ((qt, wq), (kt, wk)):
                            ps = psA.tile([128, 512], F32, tag="qkps", bufs=4)
                            for k in range(KC):
                                nc.tensor.matmul(
                                    ps[:], w[k][:, h * 128:(h + 1) * 128],
                                    xt[k][:, tsl],
                                    start=(k == 0), stop=(k == KC - 1))
                            nc.vector.tensor_copy(dst[h][:, tsl], ps[:])

                # V s-major: [t=128, dv=512] per t-chunk
                for t in range(SC):
                    ps = psA.tile([128, HD], F32, tag="vps", bufs=4)
                    for k in range(KC):
                        nc.tensor.matmul(ps[:], xt[k][:, t * 128:(t + 1) * 128],
                                         wv[k][:],
                                         start=(k == 0), stop=(k == KC - 1))
                    nc.vector.tensor_copy(vt[t][:], ps[:])

            # ---------------- attention + output projection ----------------
            with tc.tile_pool(name="attn", bufs=1) as at, \
                 tc.tile_pool(name="psB", bufs=1, space="PSUM") as psB:
                wo = []
                for c in range(HPC):
                    woc = at.tile([128, T], BF16, name=f"wo{c}")
                    nc.sync.dma_start(woc[:], woT[c * 128:(c + 1) * 128, :])
                    wo.append(woc)
                ho = [at.tile([128, T], BF16, name=f"ho{h}") for h in range(HPC)]

                for h in range(HPC):
                    hsl = slice(h * 128, (h + 1) * 128)
                    for g in range(TG):
                        tsl = slice(g * 512, (g + 1) * 512)
                        e1l, e2l = [], []
                        for sp in range(SC // 2):  # s-chunk pairs
                            s1 = psB.tile([128, 1024], F32, tag="s1")
                            s2 = psB.tile([128, 1024], F32, tag="s2")
                            for hf in range(2):
                                ssl = slice((2 * sp + hf) * 128,
                                            (2 * sp + hf + 1) * 128)
                                osl = slice(hf * 512, (hf + 1) * 512)
                                nc.tensor.matmul(s1[:, osl], kt[h][0:64, ssl],
                                                 qt[h][0:64, tsl],
                                                 start=True, stop=True)
                                nc.tensor.matmul(s2[:, osl], kt[h][64:128, ssl],
                                                 qt[h][64:128, tsl],
                                                 start=True, stop=True)
                            e1 = at.tile([128, 1024], BF16, tag="e1", bufs=10)
                            e2 = at.tile([128, 1024], BF16, tag="e2", bufs=10)
                            nc.scalar.activation(e1[:], s1[:], EXP,
                                                 scale=INV_SCALE)
                            nc.scalar.activation(e2[:], s2[:], EXP,
                                                 scale=INV_SCALE)
                            e1l.append(e1)
                            e2l.append(e2)

                        # fold E chunk-pairs on GpSimd so the denominator
                        # matmuls stream half the columns
                        f1l, f2l = [], []
                        for j in range(SC // 4):
                            f1 = at.tile([128, 1024], BF16, tag="f1", bufs=5)
                            f2 = at.tile([128, 1024], BF16, tag="f2", bufs=5)
                            nc.gpsimd.tensor_add(f1[:], e1l[j][:],
                                                 e1l[j + SC // 4][:])
                            nc.gpsimd.tensor_add(f2[:], e2l[j][:],
                                                 e2l[j + SC // 4][:])
                            f1l.append(f1)
                            f2l.append(f2)

                        p1 = psB.tile([128, 512], F32, tag="p1")
                        p2 = psB.tile([128, 512], F32, tag="p2")
                        sm1 = psB.tile([1, 512], F32, tag="smr", bufs=2)
                        sm2 = psB.tile([1, 512], F32, tag="smr", bufs=2)
                        for sp in range(SC // 2):
                            for hf in range(2):
                                s = 2 * sp + hf
                                osl = slice(hf * 512, (hf + 1) * 512)
                                st_, sp_ = (s == 0), (s == SC - 1)
                                nc.tensor.matmul(p1[:], vt[s][:, hsl],
                                                 e1l[sp][:, osl],
                                                 start=st_, stop=sp_)
                                nc.tensor.matmul(p2[:], vt[s][:, hsl],
                                                 e2l[sp][:, osl],
                                                 start=st_, stop=sp_)
                        for j in range(SC // 4):
                            for hf in range(2):
                                osl = slice(hf * 512, (hf + 1) * 512)
                                st_ = (j == 0 and hf == 0)
                                sp_ = (j == SC // 4 - 1 and hf == 1)
                                nc.tensor.matmul(sm1[:], ones_bf[:],
                                                 f1l[j][:, osl],
                                                 start=st_, stop=sp_)
                                nc.tensor.matmul(sm2[:], ones_bf[:],
                                                 f2l[j][:, osl],
                                                 start=st_, stop=sp_)

                        rc1 = at.tile([1, 512], F32, tag="rc1", bufs=2)
                        rc2 = at.tile([1, 512], F32, tag="rc2", bufs=2)
                        nc.vector.reciprocal(rc1[:], sm1[:])
                        nc.vector.reciprocal(rc2[:], sm2[:])
                        rb1 = at.tile([1, 512], BF16, tag="rb1", bufs=2)
                        rb2 = at.tile([1, 512], BF16, tag="rb2", bufs=2)
                        nc.vector.tensor_copy(rb1[:], rc1[:])
                        # fold sigmoid(lambda) into the map-2 reciprocal row
                        nc.vector.tensor_scalar(
                            rb2[:], rc2[:],
                            lamb_sb[0:1, h * 128:h * 128 + 1], None, ALU.mult)
                        r1 = psB.tile([128, 512], F32, tag="smr", bufs=2)
                        r2 = psB.tile([128, 512], F32, tag="smr", bufs=2)
                        nc.tensor.matmul(r1[:], ones_row[:], rb1[:],
                                         start=True, stop=True)
                        nc.tensor.matmul(r2[:], ones_row[:], rb2[:],
                                         start=True, stop=True)
                        r1s = at.tile([128, 512], F32, tag="r1s", bufs=2)
                        r2s = at.tile([128, 512], F32, tag="r2s", bufs=2)
                        nc.vector.tensor_copy(r1s[:], r1[:])
                        nc.vector.tensor_copy(r2s[:], r2[:])
                        tm1 = at.tile([128, 512], F32, tag="tm1", bufs=2)
                        tm2 = at.tile([128, 512], F32, tag="tm2", bufs=2)
                        nc.vector.tensor_mul(tm1[:], p1[:], r1s[:])
                        nc.vector.tensor_mul(tm2[:], p2[:], r2s[:])
                        nc.vector.tensor_sub(ho[h][:, tsl], tm1[:], tm2[:])

                # output projection: out_part[t, m] per [128, 512] tile
                otags = ["s1", "s2", "p1", "p2"]
                for t in range(SC):
                    for mg in range(TG):
                        po = psB.tile([128, 512], F32, tag=otags[mg])
                        for c in range(HPC):
                            nc.tensor.matmul(
                                po[:], ho[c][:, t * 128:(t + 1) * 128],
                                wo[c][:, mg * 512:(mg + 1) * 512],
                                start=(c == 0), stop=(c == HPC - 1))
                        ost = at.tile([128, 512], F32, tag="ost", bufs=4)
                        nc.scalar.copy(ost[:], po[:])
                        nc.sync.dma_start(
                            out[t * 128:(t + 1) * 128,
                                mg * 512:(mg + 1) * 512], ost[:])

    nc.compile()
    return nc


def kernel(x, W_q, W_k, W_v, W_o, lambda_param):
    x = np.asarray(x, dtype=np.float32)
    W_q = np.asarray(W_q, dtype=np.float32)
    W_k = np.asarray(W_k, dtype=np.float32)
    W_v = np.asarray(W_v, dtype=np.float32)
    W_o = np.asarray(W_o, dtype=np.float32)
    lambda_param = np.asarray(lambda_param, dtype=np.float32)

    bf = ml_dtypes.bfloat16
    lam = 1.0 / (1.0 + np.exp(-lambda_param))  # sigmoid, [H]

    in_maps = []
    for c in range(NCORES):
        b, hg = c // HPC, c % HPC
        hs = hg * HD
        in_maps.append({
            "xT": np.ascontiguousarray(x[b].T).astype(bf),
            "wqT": np.ascontiguousarray(W_q[hs:hs + HD, :].T).astype(bf),
            "wkT": np.ascontiguousarray(W_k[hs:hs + HD, :].T).astype(bf),
            "wvT": np.ascontiguousarray(W_v[hs:hs + HD, :].T).astype(bf),
            "woT": np.ascontiguousarray(W_o[:, hs:hs + HD].T).astype(bf),
            "lamb": np.repeat(lam[hs // DH:hs // DH + HPC], DH)
                      .reshape(1, HD).astype(np.float32),
        })

    if not _nc_cache:
        _nc_cache.append(_build())
    nc = _nc_cache[0]

    res = run_bass_kernel_spmd(nc, in_maps, core_ids=list(range(NCORES)))
    global last_result
    last_result = res
    outp = np.zeros((B, T, D), dtype=np.float32)
    for c in range(NCORES):
        outp[c // HPC] += res.results[c]["out"]
    return outp
